# revision 4
# baseline (speedup 1.0000x reference)
"""CrossNet kernel for Trainium2 (8 NeuronCores, pure data parallel over batch).

Math: reference computes, for i in 0..2:
    s_i = x_k @ w_i          (per-row dot)
    x_k = x * s_i + b_i + x_k
and returns the three intermediate x_k.

Flattened (by induction):  x_k = x * S_k + B_k + x, with
    S_{k+1} = S_k + s_k,  B_k = cumsum(b)[k-1],
    s_k = (1 + S_k) * c_k + d_k,  c_k = x @ w_k,  d_k = B_k @ w_k.
So the device kernel needs: 3 per-row dots of x with w_j, a tiny scalar
recurrence producing t_i = 1 + S_{i+1}, and out_i = x * t_i + cumb_i.

v3: fp16 end-to-end (tolerance 2e-2; fp16 costs ~5e-4). Engine plan per
measured rates (per [128,4096] fp16 pass): DVE STT+accum 4.4us (dots are
DVE-only -> 53us/core, the wall), DVE tensor_scalar 1.3us / TT 2.3us,
ScalarE ACT ~3.8us, PE matmul ~0.4-0.8us per 512-chunk (pstate), Pool TT
7.9us. Phase C leaves DVE: lanes 'p' = PE diag-matmul+cumb -> PSUM ->
ScalarE copy; 'g' = ScalarE mul + Pool add; 'd' = DVE ts-mul + TT add
(fast-tail for the last tile); 'v' = DVE STT. Broadcast tiles built by
DMA row-replication ('d') or PE ones-matmul + ScalarE copy ('p').
"""

import os

import numpy as np

B, N, ORDER, NCORES = 4096, 4096, 3, 8
ROWS = B // NCORES  # 512 rows per core
P = 128
NT = ROWS // P  # 4 partition-tiles per core

# lane per output index: 'p' PE+ScalarE, 'g' ScalarE+Pool, 'v' DVE STT,
# 'd' DVE tensor_scalar + tensor_tensor
LANES = os.environ.get("CK_LANES", "gpp")
LANES_LAST = os.environ.get("CK_LANES_LAST", "gpd")
# broadcast builder per tile wb0,wb1,wb2,cbb: 'd' DMA replicate, 'p' PE
BCAST = os.environ.get("CK_BCAST", "ddpdp")
# DMA queue per output index ('s' sync / 'a' scalar), and for x loads
OUT_Q = os.environ.get("CK_OUT_Q", "sas")
X_Q = os.environ.get("CK_X_Q", "s")
XBUFS = int(os.environ.get("CK_XBUFS", "3"))
SBUFS = int(os.environ.get("CK_SBUFS", "4"))
OBUFS = int(os.environ.get("CK_OBUFS", "6"))
TBUFS = int(os.environ.get("CK_TBUFS", "3"))

_prog_cache = {}


def _build_program():
    from contextlib import ExitStack

    import concourse.bacc as bacc
    import concourse.mybir as mybir
    import concourse.tile as tile

    f32 = mybir.dt.float32
    f16 = mybir.dt.float16
    Alu = mybir.AluOpType

    nc = bacc.Bacc("TRN2")
    xs = nc.dram_tensor("xs", [ROWS, N], f16, kind="ExternalInput")
    wr = nc.dram_tensor("wr", [ORDER, N], f16, kind="ExternalInput")
    cb = nc.dram_tensor("cb", [ORDER, N], f16, kind="ExternalInput")
    dd = nc.dram_tensor("dd", [P, ORDER], f32, kind="ExternalInput")
    eye = nc.dram_tensor("eye", [P, P], f16, kind="ExternalInput")
    out = nc.dram_tensor("out", [ORDER, ROWS, N], f16, kind="ExternalOutput")

    HALF = 2048  # psum tile free size (4 banks at fp32)

    lanes_by_tile = [LANES] * (NT - 1) + [LANES_LAST]
    need_cbb = sorted(
        {i for ls in lanes_by_tile for i, c in enumerate(ls) if c in "gvd"}
    )

    def q_eng(c):
        return nc.scalar if c == "a" else nc.sync

    with ExitStack() as ctx:
        tc = ctx.enter_context(tile.TileContext(nc))
        consts = ctx.enter_context(tc.tile_pool(name="consts", bufs=1))
        xpool = ctx.enter_context(tc.tile_pool(name="xpool", bufs=XBUFS))
        small = ctx.enter_context(tc.tile_pool(name="small", bufs=SBUFS))
        opool = ctx.enter_context(tc.tile_pool(name="opool", bufs=OBUFS))
        tpool = ctx.enter_context(tc.tile_pool(name="tpool", bufs=TBUFS))
        psum = ctx.enter_context(tc.tile_pool(name="psum", bufs=2, space="PSUM"))
        scratchpool = ctx.enter_context(tc.tile_pool(name="scratch", bufs=1))

        # w and cumb rows packed at partition bases {0, 32, 64} — the only
        # bases matmul operands may start at. One all-ones tile serves as the
        # broadcast lhsT at any of those bases. Staged in opool slots (fully
        # consumed by setup before the first ob is needed).
        wpack = opool.tile([2 * 32 + 1, N], f16, tag="ob")
        cpack = opool.tile([2 * 32 + 1, N], f16, tag="ob")
        for j in range(ORDER):
            nc.scalar.dma_start(out=wpack[32 * j : 32 * j + 1, :], in_=wr[j : j + 1, :])
            nc.scalar.dma_start(out=cpack[32 * j : 32 * j + 1, :], in_=cb[j : j + 1, :])
        dd_t = consts.tile([P, ORDER], f32, tag="dd")
        nc.scalar.dma_start(out=dd_t, in_=dd[:, :])
        eye_t = consts.tile([P, P], f16, tag="eye")
        nc.scalar.dma_start(out=eye_t, in_=eye[:, :])
        opack = consts.tile([2 * 32 + 1, P], f16, tag="opack")
        nc.vector.memset(opack, 1.0)

        def row_of(pack, j):
            return pack[32 * j : 32 * j + 1, :]

        def one_row(j):
            return opack[32 * j : 32 * j + 1, :]

        def pe_broadcast(dst, pack, j):
            # dst[128, N] = broadcast of pack row j via ones-matmul.
            for h in range(N // HALF):
                pt = psum.tile([P, HALF], f32, tag="ps")
                for q in range(HALF // 512):
                    sl = slice(h * HALF + q * 512, h * HALF + (q + 1) * 512)
                    nc.tensor.matmul(
                        pt[:, q * 512 : (q + 1) * 512],
                        lhsT=one_row(j),
                        rhs=row_of(pack, j)[:, sl],
                        start=True,
                        stop=True,
                    )
                nc.scalar.copy(dst[:, h * HALF : (h + 1) * HALF], pt)

        wb = [
            consts.tile([P, N], f16, tag=f"wb{j}", name=f"wb{j}")
            for j in range(ORDER)
        ]
        cbb = {
            i: consts.tile([P, N], f16, tag=f"cbb{i}", name=f"cbb{i}")
            for i in need_cbb
        }
        # build broadcast tiles; DMA replication goes on the scalar queue so
        # it overlaps the x loads on the sync queue during lead-in.
        for bi, (dst, src_dram, pack, j) in enumerate(
            [(wb[j], wr, wpack, j) for j in range(ORDER)]
            + [(cbb[i], cb, cpack, i) for i in need_cbb]
        ):
            mode = BCAST[bi] if bi < len(BCAST) else "p"
            if mode == "d":
                nc.scalar.dma_start(
                    out=dst, in_=src_dram[j : j + 1, :].partition_broadcast(P)
                )
            else:
                pe_broadcast(dst, pack, j)

        for k in range(NT):
            lanes = lanes_by_tile[k]
            rows = slice(k * P, (k + 1) * P)
            x_t = xpool.tile([P, N], f16, tag="x")
            q_eng(X_Q).dma_start(out=x_t, in_=xs[rows, :])

            scratch = scratchpool.tile([P, N], f16, tag="scr")
            cs, ts = [], []
            obs = [None] * ORDER
            for j in range(ORDER):
                # dot_j = sum_n x * w_j (fused multiply+reduce on DVE)
                cj = small.tile([P, 1], f32, tag=f"c{j}")
                nc.vector.scalar_tensor_tensor(
                    out=scratch,
                    in0=x_t,
                    scalar=1.0,
                    in1=wb[j],
                    op0=Alu.mult,
                    op1=Alu.mult,
                    accum_out=cj,
                )
                cs.append(cj)
                # recurrence step -> ts[j] available right after dot j
                if j == 0:
                    t1 = small.tile([P, 1], f32, tag="t0")
                    nc.vector.tensor_scalar_add(t1, cs[0], 1.0)
                    ts.append(t1)
                else:
                    si = small.tile([P, 1], f32, tag=f"s{j}")
                    nc.vector.tensor_scalar(
                        out=si,
                        in0=cs[j],
                        scalar1=ts[j - 1],
                        scalar2=dd_t[:, j : j + 1],
                        op0=Alu.mult,
                        op1=Alu.add,
                    )
                    ti = small.tile([P, 1], f32, tag=f"t{j}")
                    nc.vector.tensor_add(ti, ts[j - 1], si)
                    ts.append(ti)

                # phase C for output j (t_j just became available)
                i = j
                ob = opool.tile([P, N], f16, tag="ob")
                lane = lanes[i]
                if lane == "v":
                    nc.vector.scalar_tensor_tensor(
                        out=ob,
                        in0=x_t,
                        scalar=ts[i],
                        in1=cbb[i],
                        op0=Alu.mult,
                        op1=Alu.add,
                    )
                elif lane == "d":
                    tmp = tpool.tile([P, N], f16, tag="tmp")
                    nc.vector.tensor_scalar(
                        out=tmp, in0=x_t, scalar1=ts[i], scalar2=None, op0=Alu.mult
                    )
                    nc.vector.tensor_add(ob, tmp, cbb[i])
                elif lane == "g":
                    tmp = tpool.tile([P, N], f16, tag="tmp")
                    nc.scalar.mul(tmp, x_t, ts[i])
                    nc.gpsimd.tensor_add(ob, tmp, cbb[i])
                else:  # 'p'
                    dg = small.tile([P, P], f16, tag=f"dg{i}")
                    nc.vector.tensor_scalar_mul(dg, eye_t, ts[i])
                    for h in range(N // HALF):
                        pt = psum.tile([P, HALF], f32, tag="ps")
                        for q in range(HALF // 512):
                            sl = slice(h * HALF + q * 512, h * HALF + (q + 1) * 512)
                            nc.tensor.matmul(
                                pt[:, q * 512 : (q + 1) * 512],
                                lhsT=one_row(i),
                                rhs=row_of(cpack, i)[:, sl],
                                start=True,
                                stop=False,
                            )
                        for q in range(HALF // 512):
                            sl = slice(h * HALF + q * 512, h * HALF + (q + 1) * 512)
                            nc.tensor.matmul(
                                pt[:, q * 512 : (q + 1) * 512],
                                lhsT=dg,
                                rhs=x_t[:, sl],
                                start=False,
                                stop=True,
                            )
                        nc.scalar.copy(ob[:, h * HALF : (h + 1) * HALF], pt)
                obs[i] = ob
                q_eng(OUT_Q[i]).dma_start(out=out[i, rows, :], in_=ob)

    nc.finalize()
    return nc


def _get_program():
    if "nc" not in _prog_cache:
        _prog_cache["nc"] = _build_program()
    return _prog_cache["nc"]


def _prep_inputs(x, w, b):
    x16 = np.asarray(x, dtype=np.float32).astype(np.float16)
    w_r = np.asarray(w, dtype=np.float32).reshape(ORDER, N).astype(np.float16)
    b_r = np.asarray(b, dtype=np.float32).reshape(ORDER, N)
    cumb = np.cumsum(b_r, axis=0).astype(np.float16)  # cumb[i] = b_0 + ... + b_i
    d = np.zeros(ORDER, dtype=np.float64)
    for i in range(1, ORDER):
        d[i] = cumb[i - 1].astype(np.float64) @ w_r[i].astype(np.float64)
    dd = np.tile(d.astype(np.float32)[None, :], (P, 1))
    eye = np.eye(P, dtype=np.float16)

    shared = {
        "wr": np.ascontiguousarray(w_r),
        "cb": np.ascontiguousarray(cumb),
        "dd": np.ascontiguousarray(dd),
        "eye": eye,
    }
    in_maps = []
    for c in range(NCORES):
        m = dict(shared)
        m["xs"] = np.ascontiguousarray(x16[c * ROWS : (c + 1) * ROWS, :])
        in_maps.append(m)
    return in_maps


def _run(x, w, b, trace=False):
    from concourse.bass_utils import run_bass_kernel_spmd

    nc = _get_program()
    in_maps = _prep_inputs(x, w, b)
    res = run_bass_kernel_spmd(nc, in_maps, core_ids=list(range(NCORES)), trace=trace)
    outs = [np.asarray(r["out"]) for r in res.results]  # each [ORDER, ROWS, N] f16
    full = np.concatenate(outs, axis=1)  # [ORDER, B, N]
    return (
        tuple(np.ascontiguousarray(full[i]).astype(np.float32) for i in range(ORDER)),
        res,
    )


def kernel(x, w, b):
    outs, _ = _run(x, w, b, trace=False)
    return outs


# revision 5
# speedup vs baseline: 1.0987x; 1.0987x over previous
"""CrossNet kernel for Trainium2 (8 NeuronCores, pure data parallel over batch).

Math: reference computes, for i in 0..2:
    s_i = x_k @ w_i          (per-row dot)
    x_k = x * s_i + b_i + x_k
and returns the three intermediate x_k.

Flattened (by induction):  x_k = x * S_k + B_k + x, with
    S_{k+1} = S_k + s_k,  B_k = cumsum(b)[k-1],
    s_k = (1 + S_k) * c_k + d_k,  c_k = x @ w_k,  d_k = B_k @ w_k.
So the device kernel needs: 3 per-row dots of x with w_j, a tiny scalar
recurrence producing t_i = 1 + S_{i+1}, and out_i = x * t_i + cumb_i.

v3: fp16 end-to-end (tolerance 2e-2; fp16 costs ~5e-4). Engine plan per
measured rates (per [128,4096] fp16 pass): DVE STT+accum 4.4us (dots are
DVE-only -> 53us/core, the wall), DVE tensor_scalar 1.3us / TT 2.3us,
ScalarE ACT ~3.8us, PE matmul ~0.4-0.8us per 512-chunk (pstate), Pool TT
7.9us. Phase C leaves DVE: lanes 'p' = PE diag-matmul+cumb -> PSUM ->
ScalarE copy; 'g' = ScalarE mul + Pool add; 'd' = DVE ts-mul + TT add
(fast-tail for the last tile); 'v' = DVE STT. Broadcast tiles built by
DMA row-replication ('d') or PE ones-matmul + ScalarE copy ('p').
"""

import os

import numpy as np

B, N, ORDER, NCORES = 4096, 4096, 3, 8
ROWS = B // NCORES  # 512 rows per core
P = 128
NT = ROWS // P  # 4 partition-tiles per core

# lane per output index: 'p' PE+ScalarE, 'g' ScalarE+Pool, 'v' DVE STT,
# 'd' DVE tensor_scalar + tensor_tensor
LANES = os.environ.get("CK_LANES", "ppp")
LANES_LAST = os.environ.get("CK_LANES_LAST", "ppd")
# broadcast builder per tile wb0,wb1,wb2,cbb: 'd' DMA replicate, 'p' PE
BCAST = os.environ.get("CK_BCAST", "rrrr")
# DMA queue per output index ('s' sync / 'a' scalar), and for x loads
OUT_Q = os.environ.get("CK_OUT_Q", "sas")
X_Q = os.environ.get("CK_X_Q", "s")
XBUFS = int(os.environ.get("CK_XBUFS", "3"))
SBUFS = int(os.environ.get("CK_SBUFS", "4"))
OBUFS = int(os.environ.get("CK_OBUFS", "6"))
TBUFS = int(os.environ.get("CK_TBUFS", "3"))

_prog_cache = {}


def _build_program():
    from contextlib import ExitStack

    import concourse.bacc as bacc
    import concourse.mybir as mybir
    import concourse.tile as tile

    f32 = mybir.dt.float32
    f16 = mybir.dt.float16
    Alu = mybir.AluOpType

    nc = bacc.Bacc("TRN2")
    xs = nc.dram_tensor("xs", [ROWS, N], f16, kind="ExternalInput")
    wr = nc.dram_tensor("wr", [ORDER, N], f16, kind="ExternalInput")
    cb = nc.dram_tensor("cb", [ORDER, N], f16, kind="ExternalInput")
    wrep = nc.dram_tensor("wrep", [ORDER, P, N], f16, kind="ExternalInput")
    cbrep = nc.dram_tensor("cbrep", [ORDER, P, N], f16, kind="ExternalInput")
    dd = nc.dram_tensor("dd", [P, ORDER], f32, kind="ExternalInput")
    eye = nc.dram_tensor("eye", [P, P], f16, kind="ExternalInput")
    out = nc.dram_tensor("out", [ORDER, ROWS, N], f16, kind="ExternalOutput")

    HALF = 2048  # psum tile free size (4 banks at fp32)

    lanes_by_tile = [LANES] * (NT - 1) + [LANES_LAST]
    need_cbb = sorted(
        {i for ls in lanes_by_tile for i, c in enumerate(ls) if c in "gvd"}
    )

    def q_eng(c):
        return nc.scalar if c == "a" else nc.sync

    with ExitStack() as ctx:
        tc = ctx.enter_context(tile.TileContext(nc))
        consts = ctx.enter_context(tc.tile_pool(name="consts", bufs=1))
        xpool = ctx.enter_context(tc.tile_pool(name="xpool", bufs=XBUFS))
        small = ctx.enter_context(tc.tile_pool(name="small", bufs=SBUFS))
        opool = ctx.enter_context(tc.tile_pool(name="opool", bufs=OBUFS))
        tpool = ctx.enter_context(tc.tile_pool(name="tpool", bufs=TBUFS))
        psum = ctx.enter_context(tc.tile_pool(name="psum", bufs=2, space="PSUM"))
        scratchpool = ctx.enter_context(tc.tile_pool(name="scratch", bufs=1))

        # w and cumb rows packed at partition bases {0, 32, 64} — the only
        # bases matmul operands may start at. One all-ones tile serves as the
        # broadcast lhsT at any of those bases. Staged in opool slots (fully
        # consumed by setup before the first ob is needed).
        wpack = opool.tile([2 * 32 + 1, N], f16, tag="ob")
        cpack = opool.tile([2 * 32 + 1, N], f16, tag="ob")
        for j in range(ORDER):
            nc.scalar.dma_start(out=wpack[32 * j : 32 * j + 1, :], in_=wr[j : j + 1, :])
            nc.scalar.dma_start(out=cpack[32 * j : 32 * j + 1, :], in_=cb[j : j + 1, :])
        dd_t = consts.tile([P, ORDER], f32, tag="dd")
        nc.scalar.dma_start(out=dd_t, in_=dd[:, :])
        eye_t = consts.tile([P, P], f16, tag="eye")
        nc.scalar.dma_start(out=eye_t, in_=eye[:, :])
        opack = consts.tile([2 * 32 + 1, P], f16, tag="opack")
        nc.vector.memset(opack, 1.0)

        def row_of(pack, j):
            return pack[32 * j : 32 * j + 1, :]

        def one_row(j):
            return opack[32 * j : 32 * j + 1, :]

        def pe_broadcast(dst, pack, j):
            # dst[128, N] = broadcast of pack row j via ones-matmul.
            for h in range(N // HALF):
                pt = psum.tile([P, HALF], f32, tag="ps")
                for q in range(HALF // 512):
                    sl = slice(h * HALF + q * 512, h * HALF + (q + 1) * 512)
                    nc.tensor.matmul(
                        pt[:, q * 512 : (q + 1) * 512],
                        lhsT=one_row(j),
                        rhs=row_of(pack, j)[:, sl],
                        start=True,
                        stop=True,
                    )
                nc.scalar.copy(dst[:, h * HALF : (h + 1) * HALF], pt)

        wb = [
            consts.tile([P, N], f16, tag=f"wb{j}", name=f"wb{j}")
            for j in range(ORDER)
        ]
        cbb = {
            i: consts.tile([P, N], f16, tag=f"cbb{i}", name=f"cbb{i}")
            for i in need_cbb
        }
        # build broadcast tiles; DMA replication goes on the scalar queue so
        # it overlaps the x loads on the sync queue during lead-in.
        for bi, (dst, src_dram, rep_dram, pack, j) in enumerate(
            [(wb[j], wr, wrep, wpack, j) for j in range(ORDER)]
            + [(cbb[i], cb, cbrep, cpack, i) for i in need_cbb]
        ):
            mode = BCAST[bi] if bi < len(BCAST) else "p"
            if mode == "d":
                nc.scalar.dma_start(
                    out=dst, in_=src_dram[j : j + 1, :].partition_broadcast(P)
                )
            elif mode == "r":
                nc.scalar.dma_start(out=dst, in_=rep_dram[j])
            else:
                pe_broadcast(dst, pack, j)

        for k in range(NT):
            lanes = lanes_by_tile[k]
            rows = slice(k * P, (k + 1) * P)
            x_t = xpool.tile([P, N], f16, tag="x")
            q_eng(X_Q).dma_start(out=x_t, in_=xs[rows, :])

            scratch = scratchpool.tile([P, N], f16, tag="scr")
            cs, ts = [], []
            obs = [None] * ORDER
            for j in range(ORDER):
                # dot_j = sum_n x * w_j (fused multiply+reduce on DVE)
                cj = small.tile([P, 1], f32, tag=f"c{j}")
                nc.vector.scalar_tensor_tensor(
                    out=scratch,
                    in0=x_t,
                    scalar=1.0,
                    in1=wb[j],
                    op0=Alu.mult,
                    op1=Alu.mult,
                    accum_out=cj,
                )
                cs.append(cj)
                # recurrence step -> ts[j] available right after dot j
                if j == 0:
                    t1 = small.tile([P, 1], f32, tag="t0")
                    nc.vector.tensor_scalar_add(t1, cs[0], 1.0)
                    ts.append(t1)
                else:
                    si = small.tile([P, 1], f32, tag=f"s{j}")
                    nc.vector.tensor_scalar(
                        out=si,
                        in0=cs[j],
                        scalar1=ts[j - 1],
                        scalar2=dd_t[:, j : j + 1],
                        op0=Alu.mult,
                        op1=Alu.add,
                    )
                    ti = small.tile([P, 1], f32, tag=f"t{j}")
                    nc.vector.tensor_add(ti, ts[j - 1], si)
                    ts.append(ti)

                # phase C for output j (t_j just became available)
                i = j
                ob = opool.tile([P, N], f16, tag="ob")
                lane = lanes[i]
                if lane == "v":
                    nc.vector.scalar_tensor_tensor(
                        out=ob,
                        in0=x_t,
                        scalar=ts[i],
                        in1=cbb[i],
                        op0=Alu.mult,
                        op1=Alu.add,
                    )
                elif lane == "d":
                    tmp = tpool.tile([P, N], f16, tag="tmp")
                    nc.vector.tensor_scalar(
                        out=tmp, in0=x_t, scalar1=ts[i], scalar2=None, op0=Alu.mult
                    )
                    nc.vector.tensor_add(ob, tmp, cbb[i])
                elif lane == "g":
                    tmp = tpool.tile([P, N], f16, tag="tmp")
                    nc.scalar.mul(tmp, x_t, ts[i])
                    nc.gpsimd.tensor_add(ob, tmp, cbb[i])
                else:  # 'p'
                    dg = small.tile([P, P], f16, tag=f"dg{i}")
                    nc.vector.tensor_scalar_mul(dg, eye_t, ts[i])
                    for h in range(N // HALF):
                        pt = psum.tile([P, HALF], f32, tag="ps")
                        for q in range(HALF // 512):
                            sl = slice(h * HALF + q * 512, h * HALF + (q + 1) * 512)
                            nc.tensor.matmul(
                                pt[:, q * 512 : (q + 1) * 512],
                                lhsT=one_row(i),
                                rhs=row_of(cpack, i)[:, sl],
                                start=True,
                                stop=False,
                            )
                        for q in range(HALF // 512):
                            sl = slice(h * HALF + q * 512, h * HALF + (q + 1) * 512)
                            nc.tensor.matmul(
                                pt[:, q * 512 : (q + 1) * 512],
                                lhsT=dg,
                                rhs=x_t[:, sl],
                                start=False,
                                stop=True,
                            )
                        nc.scalar.copy(ob[:, h * HALF : (h + 1) * HALF], pt)
                obs[i] = ob
                q_eng(OUT_Q[i]).dma_start(out=out[i, rows, :], in_=ob)

    nc.finalize()
    return nc


def _get_program():
    if "nc" not in _prog_cache:
        _prog_cache["nc"] = _build_program()
    return _prog_cache["nc"]


def _prep_inputs(x, w, b):
    x16 = np.asarray(x, dtype=np.float32).astype(np.float16)
    w_r = np.asarray(w, dtype=np.float32).reshape(ORDER, N).astype(np.float16)
    b_r = np.asarray(b, dtype=np.float32).reshape(ORDER, N)
    cumb = np.cumsum(b_r, axis=0).astype(np.float16)  # cumb[i] = b_0 + ... + b_i
    d = np.zeros(ORDER, dtype=np.float64)
    for i in range(1, ORDER):
        d[i] = cumb[i - 1].astype(np.float64) @ w_r[i].astype(np.float64)
    dd = np.tile(d.astype(np.float32)[None, :], (P, 1))
    eye = np.eye(P, dtype=np.float16)

    shared = {
        "wr": np.ascontiguousarray(w_r),
        "cb": np.ascontiguousarray(cumb),
        "wrep": np.ascontiguousarray(np.broadcast_to(w_r[:, None, :], (ORDER, P, N))),
        "cbrep": np.ascontiguousarray(np.broadcast_to(cumb[:, None, :], (ORDER, P, N))),
        "dd": np.ascontiguousarray(dd),
        "eye": eye,
    }
    in_maps = []
    for c in range(NCORES):
        m = dict(shared)
        m["xs"] = np.ascontiguousarray(x16[c * ROWS : (c + 1) * ROWS, :])
        in_maps.append(m)
    return in_maps


def _run(x, w, b, trace=False):
    from concourse.bass_utils import run_bass_kernel_spmd

    nc = _get_program()
    in_maps = _prep_inputs(x, w, b)
    res = run_bass_kernel_spmd(nc, in_maps, core_ids=list(range(NCORES)), trace=trace)
    outs = [np.asarray(r["out"]) for r in res.results]  # each [ORDER, ROWS, N] f16
    full = np.concatenate(outs, axis=1)  # [ORDER, B, N]
    return (
        tuple(np.ascontiguousarray(full[i]).astype(np.float32) for i in range(ORDER)),
        res,
    )


def kernel(x, w, b):
    outs, _ = _run(x, w, b, trace=False)
    return outs


# revision 6
# speedup vs baseline: 1.1168x; 1.0165x over previous
"""CrossNet kernel for Trainium2 (8 NeuronCores, pure data parallel over batch).

Math: reference computes, for i in 0..2:
    s_i = x_k @ w_i          (per-row dot)
    x_k = x * s_i + b_i + x_k
and returns the three intermediate x_k.

Flattened (by induction):  x_k = x * S_k + B_k + x, with
    S_{k+1} = S_k + s_k,  B_k = cumsum(b)[k-1],
    s_k = (1 + S_k) * c_k + d_k,  c_k = x @ w_k,  d_k = B_k @ w_k.
Device work: 3 per-row dots of x with w_j, a tiny scalar recurrence
producing t_i = 1 + S_{i+1}, and out_i = x * t_i + cumb_i.

v6, engine plan from measured fp16 rates per [128,4096] pass:
  DVE:  STT+accum 4.42us (1x), TT 2.29us (2x_1p), tensor_scalar 1.29us
        (4x_2p); any GPSIMD activity poisons concurrent DVE ~2.7x -> Pool
        unused.
  Scal: ACT ~3.8us (supports accum_out reduce), PSUM copy 1.96us/half.
  PE:   ~590ns per 512-col fp16 matmul (pstate); out via diag(t)-matmul +
        ones x cumb accumulate = 16 MM -> ~7.3us/out + 2 copies.
Dots 'v' = DVE STT+accum; 's' = DVE TT-mult (2x) + ScalarE ACT-reduce.
Outs 'p' = PE+ScalarE-copy; 'd' = DVE ts-mul (4x) + TT-add (2x);
     'v' = DVE STT; 'g' = ScalarE-mul + Pool-add (avoid: poisons DVE).
Broadcast wb/cbb tiles arrive host-replicated via contiguous DMA on the
scalar queue, ordered so wb0 lands ~3us in (dots start immediately).
fp16 end-to-end: tolerance is 2e-2, fp16 costs ~5e-4.
"""

import os

import numpy as np

B, N, ORDER, NCORES = 4096, 4096, 3, 8
ROWS = B // NCORES  # 512 rows per core
P = 128
NT = ROWS // P  # 4 partition-tiles per core

# 12-char strings (tile-major: tile0 j0,j1,j2, tile1 j0,...) selecting the
# implementation of each dot and each output; 3-char strings repeat per tile.
DOTS = os.environ.get("CK_DOTS", "vsv" "svs" "vsv" "svs")
LANES = os.environ.get("CK_LANES", "pdp" "pdp" "pdp" "ppd")
# DMA queue per output index ('s' sync / 'a' scalar), and for x loads
OUT_Q = os.environ.get("CK_OUT_Q", "sas")
X_Q = os.environ.get("CK_X_Q", "s")
XBUFS = int(os.environ.get("CK_XBUFS", "3"))
SBUFS = int(os.environ.get("CK_SBUFS", "4"))
OBUFS = int(os.environ.get("CK_OBUFS", "6"))
TBUFS = int(os.environ.get("CK_TBUFS", "3"))
PBUFS = int(os.environ.get("CK_PBUFS", "2"))


def _expand(s):
    s = "".join(c for c in s if not c.isspace())
    if len(s) == ORDER:
        s = s * NT
    assert len(s) == ORDER * NT, s
    return s


_prog_cache = {}


def _build_program():
    from contextlib import ExitStack

    import concourse.bacc as bacc
    import concourse.mybir as mybir
    import concourse.tile as tile

    f32 = mybir.dt.float32
    f16 = mybir.dt.float16
    Alu = mybir.AluOpType
    Act = mybir.ActivationFunctionType

    dots = _expand(DOTS)
    lanes = _expand(LANES)

    nc = bacc.Bacc("TRN2")
    xs = nc.dram_tensor("xs", [ROWS, N], f16, kind="ExternalInput")
    wrep = nc.dram_tensor("wrep", [ORDER, P, N], f16, kind="ExternalInput")
    cbrep = nc.dram_tensor("cbrep", [ORDER, P, N], f16, kind="ExternalInput")
    cb = nc.dram_tensor("cb", [ORDER, N], f16, kind="ExternalInput")
    dd = nc.dram_tensor("dd", [P, ORDER], f32, kind="ExternalInput")
    eye = nc.dram_tensor("eye", [P, P], f16, kind="ExternalInput")
    out = nc.dram_tensor("out", [ORDER, ROWS, N], f16, kind="ExternalOutput")

    HALF = 2048  # psum tile free size (4 banks at fp32)

    need_cbb = sorted(
        {
            q
            for k in range(NT)
            for q, c in enumerate(lanes[3 * k : 3 * k + 3])
            if c in "gvd"
        }
    )
    any_pe_out = any(c == "p" for c in lanes)

    def q_eng(c):
        return nc.scalar if c == "a" else nc.sync

    with ExitStack() as ctx:
        tc = ctx.enter_context(tile.TileContext(nc))
        consts = ctx.enter_context(tc.tile_pool(name="consts", bufs=1))
        xpool = ctx.enter_context(tc.tile_pool(name="xpool", bufs=XBUFS))
        small = ctx.enter_context(tc.tile_pool(name="small", bufs=SBUFS))
        opool = ctx.enter_context(tc.tile_pool(name="opool", bufs=OBUFS))
        tpool = ctx.enter_context(tc.tile_pool(name="tpool", bufs=TBUFS))
        psum = ctx.enter_context(tc.tile_pool(name="psum", bufs=PBUFS, space="PSUM"))
        scratchpool = ctx.enter_context(tc.tile_pool(name="scratch", bufs=1))

        wb = [
            consts.tile([P, N], f16, tag=f"wb{j}", name=f"wb{j}")
            for j in range(ORDER)
        ]
        cbb = {
            i: consts.tile([P, N], f16, tag=f"cbb{i}", name=f"cbb{i}")
            for i in need_cbb
        }
        dd_t = consts.tile([P, ORDER], f32, tag="dd")
        eye_t = consts.tile([P, P], f16, tag="eye")

        # Scalar-queue DMA order is the lead-in critical path: wb0 first so
        # dot0 can start ~3us in, then dd (needed by the first recurrence),
        # then the rest. cpack rows feed the PE-lane cumb matmuls.
        nc.scalar.dma_start(out=wb[0], in_=wrep[0])
        nc.scalar.dma_start(out=dd_t, in_=dd[:, :])
        nc.scalar.dma_start(out=wb[1], in_=wrep[1])
        cpack = None
        if any_pe_out:
            cpack = consts.tile([2 * 32 + 1, N], f16, tag="cpack")
            for j in range(ORDER):
                nc.scalar.dma_start(
                    out=cpack[32 * j : 32 * j + 1, :], in_=cb[j : j + 1, :]
                )
            nc.scalar.dma_start(out=eye_t, in_=eye[:, :])
        nc.scalar.dma_start(out=wb[2], in_=wrep[2])
        for i in need_cbb:
            nc.scalar.dma_start(out=cbb[i], in_=cbrep[i])
        opack = consts.tile([2 * 32 + 1, P], f16, tag="opack")
        nc.vector.memset(opack, 1.0)

        def row_of(pack, j):
            return pack[32 * j : 32 * j + 1, :]

        def one_row(j):
            return opack[32 * j : 32 * j + 1, :]

        for k in range(NT):
            rows = slice(k * P, (k + 1) * P)
            x_t = xpool.tile([P, N], f16, tag="x")
            q_eng(X_Q).dma_start(out=x_t, in_=xs[rows, :])

            scratch = scratchpool.tile([P, N], f16, tag="scr")
            cs, ts = [], []
            for j in range(ORDER):
                # dot_j = sum_n x * w_j
                cj = small.tile([P, 1], f32, tag=f"c{j}")
                if dots[3 * k + j] == "s":
                    # DVE TT-mult at 2x, reduce on ScalarE ACT accumulator
                    prod = tpool.tile([P, N], f16, tag="prod")
                    nc.vector.tensor_tensor(prod, x_t, wb[j], Alu.mult)
                    nc.scalar.activation(prod, prod, Act.Copy, accum_out=cj)
                else:
                    nc.vector.scalar_tensor_tensor(
                        out=scratch,
                        in0=x_t,
                        scalar=1.0,
                        in1=wb[j],
                        op0=Alu.mult,
                        op1=Alu.mult,
                        accum_out=cj,
                    )
                cs.append(cj)
                # recurrence step -> ts[j] available right after dot j
                if j == 0:
                    t1 = small.tile([P, 1], f32, tag="t0")
                    nc.vector.tensor_scalar_add(t1, cs[0], 1.0)
                    ts.append(t1)
                else:
                    si = small.tile([P, 1], f32, tag=f"s{j}")
                    nc.vector.tensor_scalar(
                        out=si,
                        in0=cs[j],
                        scalar1=ts[j - 1],
                        scalar2=dd_t[:, j : j + 1],
                        op0=Alu.mult,
                        op1=Alu.add,
                    )
                    ti = small.tile([P, 1], f32, tag=f"t{j}")
                    nc.vector.tensor_add(ti, ts[j - 1], si)
                    ts.append(ti)

                # phase C for output j (t_j just became available)
                i = j
                ob = opool.tile([P, N], f16, tag="ob")
                lane = lanes[3 * k + i]
                if lane == "v":
                    nc.vector.scalar_tensor_tensor(
                        out=ob,
                        in0=x_t,
                        scalar=ts[i],
                        in1=cbb[i],
                        op0=Alu.mult,
                        op1=Alu.add,
                    )
                elif lane == "d":
                    nc.vector.tensor_scalar(
                        out=ob, in0=x_t, scalar1=ts[i], scalar2=None, op0=Alu.mult
                    )
                    nc.vector.tensor_add(ob, ob, cbb[i])
                elif lane == "g":
                    tmp = tpool.tile([P, N], f16, tag="prod")
                    nc.scalar.mul(tmp, x_t, ts[i])
                    nc.gpsimd.tensor_add(ob, tmp, cbb[i])
                else:  # 'p'
                    dg = small.tile([P, P], f16, tag=f"dg{i}")
                    nc.vector.tensor_scalar_mul(dg, eye_t, ts[i])
                    for h in range(N // HALF):
                        pt = psum.tile([P, HALF], f32, tag="ps")
                        for q in range(HALF // 512):
                            sl = slice(h * HALF + q * 512, h * HALF + (q + 1) * 512)
                            nc.tensor.matmul(
                                pt[:, q * 512 : (q + 1) * 512],
                                lhsT=one_row(i),
                                rhs=row_of(cpack, i)[:, sl],
                                start=True,
                                stop=False,
                            )
                        for q in range(HALF // 512):
                            sl = slice(h * HALF + q * 512, h * HALF + (q + 1) * 512)
                            nc.tensor.matmul(
                                pt[:, q * 512 : (q + 1) * 512],
                                lhsT=dg,
                                rhs=x_t[:, sl],
                                start=False,
                                stop=True,
                            )
                        nc.scalar.copy(ob[:, h * HALF : (h + 1) * HALF], pt)
                q_eng(OUT_Q[i]).dma_start(out=out[i, rows, :], in_=ob)

    nc.finalize()
    return nc


def _get_program():
    if "nc" not in _prog_cache:
        _prog_cache["nc"] = _build_program()
    return _prog_cache["nc"]


def _prep_inputs(x, w, b):
    x16 = np.asarray(x, dtype=np.float32).astype(np.float16)
    w_r = np.asarray(w, dtype=np.float32).reshape(ORDER, N).astype(np.float16)
    b_r = np.asarray(b, dtype=np.float32).reshape(ORDER, N)
    cumb = np.cumsum(b_r, axis=0).astype(np.float16)  # cumb[i] = b_0 + ... + b_i
    d = np.zeros(ORDER, dtype=np.float64)
    for i in range(1, ORDER):
        d[i] = cumb[i - 1].astype(np.float64) @ w_r[i].astype(np.float64)
    dd = np.tile(d.astype(np.float32)[None, :], (P, 1))
    eye = np.eye(P, dtype=np.float16)

    shared = {
        "cb": np.ascontiguousarray(cumb),
        "wrep": np.ascontiguousarray(np.broadcast_to(w_r[:, None, :], (ORDER, P, N))),
        "cbrep": np.ascontiguousarray(
            np.broadcast_to(cumb[:, None, :], (ORDER, P, N))
        ),
        "dd": np.ascontiguousarray(dd),
        "eye": eye,
    }
    in_maps = []
    for c in range(NCORES):
        m = dict(shared)
        m["xs"] = np.ascontiguousarray(x16[c * ROWS : (c + 1) * ROWS, :])
        in_maps.append(m)
    return in_maps


def _run(x, w, b, trace=False):
    from concourse.bass_utils import run_bass_kernel_spmd

    nc = _get_program()
    in_maps = _prep_inputs(x, w, b)
    res = run_bass_kernel_spmd(nc, in_maps, core_ids=list(range(NCORES)), trace=trace)
    outs = [np.asarray(r["out"]) for r in res.results]  # each [ORDER, ROWS, N] f16
    full = np.concatenate(outs, axis=1)  # [ORDER, B, N]
    return (
        tuple(np.ascontiguousarray(full[i]).astype(np.float32) for i in range(ORDER)),
        res,
    )


def kernel(x, w, b):
    outs, _ = _run(x, w, b, trace=False)
    return outs


# revision 7
# speedup vs baseline: 1.2191x; 1.0915x over previous
"""CrossNet kernel for Trainium2 (8 NeuronCores, pure data parallel over batch).

Math: reference computes, for i in 0..2:
    s_i = x_k @ w_i          (per-row dot)
    x_k = x * s_i + b_i + x_k
and returns the three intermediate x_k.

Flattened (by induction):  x_k = x * S_k + B_k + x, with
    S_{k+1} = S_k + s_k,  B_k = cumsum(b)[k-1],
    s_k = (1 + S_k) * c_k + d_k,  c_k = x @ w_k,  d_k = B_k @ w_k.
Device work: 3 per-row dots of x with w_j, a tiny scalar recurrence
producing t_i = 1 + S_{i+1}, and out_i = x * t_i + cumb_i.

v6, engine plan from measured fp16 rates per [128,4096] pass:
  DVE:  STT+accum 4.42us (1x), TT 2.29us (2x_1p), tensor_scalar 1.29us
        (4x_2p); any GPSIMD activity poisons concurrent DVE ~2.7x -> Pool
        unused.
  Scal: ACT ~3.8us (supports accum_out reduce), PSUM copy 1.96us/half.
  PE:   ~590ns per 512-col fp16 matmul (pstate); out via diag(t)-matmul +
        ones x cumb accumulate = 16 MM -> ~7.3us/out + 2 copies.
Dots 'v' = DVE STT+accum; 's' = DVE TT-mult (2x) + ScalarE ACT-reduce.
Outs 'p' = PE+ScalarE-copy; 'd' = DVE ts-mul (4x) + TT-add (2x);
     'v' = DVE STT; 'g' = ScalarE-mul + Pool-add (avoid: poisons DVE).
Broadcast wb/cbb tiles arrive host-replicated via contiguous DMA on the
scalar queue, ordered so wb0 lands ~3us in (dots start immediately).
fp16 end-to-end: tolerance is 2e-2, fp16 costs ~5e-4.
"""

import os

import numpy as np

B, N, ORDER, NCORES = 4096, 4096, 3, 8
ROWS = B // NCORES  # 512 rows per core
P = 128
NT = ROWS // P  # 4 partition-tiles per core

# 12-char strings (tile-major: tile0 j0,j1,j2, tile1 j0,...) selecting the
# implementation of each dot and each output; 3-char strings repeat per tile.
DOTS = os.environ.get("CK_DOTS", "vsv" "svs" "vsv" "vvv")
LANES = os.environ.get("CK_LANES", "pdp" "pdp" "pdp" "ddd")
# DMA queue per output index ('s' sync / 'a' scalar), and for x loads
OUT_Q = os.environ.get("CK_OUT_Q", "sas")
X_Q = os.environ.get("CK_X_Q", "s")
XBUFS = int(os.environ.get("CK_XBUFS", "3"))
SBUFS = int(os.environ.get("CK_SBUFS", "4"))
OBUFS = int(os.environ.get("CK_OBUFS", "6"))
TBUFS = int(os.environ.get("CK_TBUFS", "3"))
PBUFS = int(os.environ.get("CK_PBUFS", "2"))


def _expand(s):
    s = "".join(c for c in s if not c.isspace())
    if len(s) == ORDER:
        s = s * NT
    assert len(s) == ORDER * NT, s
    return s


_prog_cache = {}


def _build_program():
    from contextlib import ExitStack

    import concourse.bacc as bacc
    import concourse.mybir as mybir
    import concourse.tile as tile

    f32 = mybir.dt.float32
    f16 = mybir.dt.float16
    Alu = mybir.AluOpType
    Act = mybir.ActivationFunctionType

    dots = _expand(DOTS)
    lanes = _expand(LANES)

    nc = bacc.Bacc("TRN2")
    xs = nc.dram_tensor("xs", [ROWS, N], f16, kind="ExternalInput")
    wrep = nc.dram_tensor("wrep", [ORDER, P, N], f16, kind="ExternalInput")
    cbrep = nc.dram_tensor("cbrep", [ORDER, P, N], f16, kind="ExternalInput")
    cb = nc.dram_tensor("cb", [ORDER, N], f16, kind="ExternalInput")
    dd = nc.dram_tensor("dd", [P, ORDER], f32, kind="ExternalInput")
    eye = nc.dram_tensor("eye", [P, P], f16, kind="ExternalInput")
    out = nc.dram_tensor("out", [ORDER, ROWS, N], f16, kind="ExternalOutput")

    HALF = 2048  # psum tile free size (4 banks at fp32)

    need_cbb = sorted(
        {
            q
            for k in range(NT)
            for q, c in enumerate(lanes[3 * k : 3 * k + 3])
            if c in "gvd"
        }
    )
    any_pe_out = any(c == "p" for c in lanes)

    def q_eng(c):
        return nc.scalar if c == "a" else nc.sync

    with ExitStack() as ctx:
        tc = ctx.enter_context(tile.TileContext(nc))
        consts = ctx.enter_context(tc.tile_pool(name="consts", bufs=1))
        xpool = ctx.enter_context(tc.tile_pool(name="xpool", bufs=XBUFS))
        small = ctx.enter_context(tc.tile_pool(name="small", bufs=SBUFS))
        opool = ctx.enter_context(tc.tile_pool(name="opool", bufs=OBUFS))
        tpool = ctx.enter_context(tc.tile_pool(name="tpool", bufs=TBUFS))
        psum = ctx.enter_context(tc.tile_pool(name="psum", bufs=PBUFS, space="PSUM"))
        scratchpool = ctx.enter_context(tc.tile_pool(name="scratch", bufs=1))

        wb = [
            consts.tile([P, N], f16, tag=f"wb{j}", name=f"wb{j}")
            for j in range(ORDER)
        ]
        cbb = {
            i: consts.tile([P, N], f16, tag=f"cbb{i}", name=f"cbb{i}")
            for i in need_cbb
        }
        dd_t = consts.tile([P, ORDER], f32, tag="dd")
        eye_t = consts.tile([P, P], f16, tag="eye")

        # Scalar-queue DMA order is the lead-in critical path: wb0 first so
        # dot0 can start ~3us in, then dd (needed by the first recurrence),
        # then the rest. cpack rows feed the PE-lane cumb matmuls.
        nc.scalar.dma_start(out=wb[0], in_=wrep[0])
        nc.scalar.dma_start(out=dd_t, in_=dd[:, :])
        nc.scalar.dma_start(out=wb[1], in_=wrep[1])
        cpack = None
        if any_pe_out:
            cpack = consts.tile([2 * 32 + 1, N], f16, tag="cpack")
            for j in range(ORDER):
                nc.scalar.dma_start(
                    out=cpack[32 * j : 32 * j + 1, :], in_=cb[j : j + 1, :]
                )
            nc.scalar.dma_start(out=eye_t, in_=eye[:, :])
        nc.scalar.dma_start(out=wb[2], in_=wrep[2])
        for i in need_cbb:
            nc.scalar.dma_start(out=cbb[i], in_=cbrep[i])
        opack = consts.tile([2 * 32 + 1, P], f16, tag="opack")
        nc.vector.memset(opack, 1.0)

        def row_of(pack, j):
            return pack[32 * j : 32 * j + 1, :]

        def one_row(j):
            return opack[32 * j : 32 * j + 1, :]

        def emit_dots(k, x_t):
            scratch = scratchpool.tile([P, N], f16, tag="scr")
            cs, ts = [], []
            for j in range(ORDER):
                # dot_j = sum_n x * w_j
                cj = small.tile([P, 1], f32, tag=f"c{j}")
                if dots[3 * k + j] == "s":
                    # DVE TT-mult at 2x, reduce on ScalarE ACT accumulator
                    prod = tpool.tile([P, N], f16, tag="prod")
                    nc.vector.tensor_tensor(prod, x_t, wb[j], Alu.mult)
                    nc.scalar.activation(prod, prod, Act.Copy, accum_out=cj)
                else:
                    nc.vector.scalar_tensor_tensor(
                        out=scratch,
                        in0=x_t,
                        scalar=1.0,
                        in1=wb[j],
                        op0=Alu.mult,
                        op1=Alu.mult,
                        accum_out=cj,
                    )
                cs.append(cj)
                # recurrence step -> ts[j] available right after dot j
                if j == 0:
                    t1 = small.tile([P, 1], f32, tag="t0")
                    nc.vector.tensor_scalar_add(t1, cs[0], 1.0)
                    ts.append(t1)
                else:
                    si = small.tile([P, 1], f32, tag=f"s{j}")
                    nc.vector.tensor_scalar(
                        out=si,
                        in0=cs[j],
                        scalar1=ts[j - 1],
                        scalar2=dd_t[:, j : j + 1],
                        op0=Alu.mult,
                        op1=Alu.add,
                    )
                    ti = small.tile([P, 1], f32, tag=f"t{j}")
                    nc.vector.tensor_add(ti, ts[j - 1], si)
                    ts.append(ti)
            return ts

        def emit_phase_c(k, x_t, ts):
            rows = slice(k * P, (k + 1) * P)
            for i in range(ORDER):
                ob = opool.tile([P, N], f16, tag="ob")
                lane = lanes[3 * k + i]
                if lane == "v":
                    nc.vector.scalar_tensor_tensor(
                        out=ob,
                        in0=x_t,
                        scalar=ts[i],
                        in1=cbb[i],
                        op0=Alu.mult,
                        op1=Alu.add,
                    )
                elif lane == "d":
                    nc.vector.tensor_scalar(
                        out=ob, in0=x_t, scalar1=ts[i], scalar2=None, op0=Alu.mult
                    )
                    nc.vector.tensor_add(ob, ob, cbb[i])
                elif lane == "g":
                    tmp = tpool.tile([P, N], f16, tag="prod")
                    nc.scalar.mul(tmp, x_t, ts[i])
                    nc.gpsimd.tensor_add(ob, tmp, cbb[i])
                else:  # 'p'
                    dg = small.tile([P, P], f16, tag=f"dg{i}")
                    nc.vector.tensor_scalar_mul(dg, eye_t, ts[i])
                    for h in range(N // HALF):
                        pt = psum.tile([P, HALF], f32, tag="ps")
                        for q in range(HALF // 512):
                            sl = slice(h * HALF + q * 512, h * HALF + (q + 1) * 512)
                            nc.tensor.matmul(
                                pt[:, q * 512 : (q + 1) * 512],
                                lhsT=one_row(i),
                                rhs=row_of(cpack, i)[:, sl],
                                start=True,
                                stop=False,
                            )
                        for q in range(HALF // 512):
                            sl = slice(h * HALF + q * 512, h * HALF + (q + 1) * 512)
                            nc.tensor.matmul(
                                pt[:, q * 512 : (q + 1) * 512],
                                lhsT=dg,
                                rhs=x_t[:, sl],
                                start=False,
                                stop=True,
                            )
                        nc.scalar.copy(ob[:, h * HALF : (h + 1) * HALF], pt)
                q_eng(OUT_Q[i]).dma_start(out=out[i, rows, :], in_=ob)

        # Software-pipelined emission: dots of tile k+1 are enqueued before
        # phase C of tile k, so no engine's phase-C backlog ever sits ahead
        # of the recurrence chain in a queue.
        xt_hist, ts_hist = {}, {}
        for k in range(NT):
            x_t = xpool.tile([P, N], f16, tag="x")
            q_eng(X_Q).dma_start(out=x_t, in_=xs[k * P : (k + 1) * P, :])
            xt_hist[k] = x_t
            ts_hist[k] = emit_dots(k, x_t)
            if k > 0:
                emit_phase_c(k - 1, xt_hist[k - 1], ts_hist[k - 1])
        emit_phase_c(NT - 1, xt_hist[NT - 1], ts_hist[NT - 1])

    nc.finalize()
    return nc


def _get_program():
    if "nc" not in _prog_cache:
        _prog_cache["nc"] = _build_program()
    return _prog_cache["nc"]


def _prep_inputs(x, w, b):
    x16 = np.asarray(x, dtype=np.float32).astype(np.float16)
    w_r = np.asarray(w, dtype=np.float32).reshape(ORDER, N).astype(np.float16)
    b_r = np.asarray(b, dtype=np.float32).reshape(ORDER, N)
    cumb = np.cumsum(b_r, axis=0).astype(np.float16)  # cumb[i] = b_0 + ... + b_i
    d = np.zeros(ORDER, dtype=np.float64)
    for i in range(1, ORDER):
        d[i] = cumb[i - 1].astype(np.float64) @ w_r[i].astype(np.float64)
    dd = np.tile(d.astype(np.float32)[None, :], (P, 1))
    eye = np.eye(P, dtype=np.float16)

    shared = {
        "cb": np.ascontiguousarray(cumb),
        "wrep": np.ascontiguousarray(np.broadcast_to(w_r[:, None, :], (ORDER, P, N))),
        "cbrep": np.ascontiguousarray(
            np.broadcast_to(cumb[:, None, :], (ORDER, P, N))
        ),
        "dd": np.ascontiguousarray(dd),
        "eye": eye,
    }
    in_maps = []
    for c in range(NCORES):
        m = dict(shared)
        m["xs"] = np.ascontiguousarray(x16[c * ROWS : (c + 1) * ROWS, :])
        in_maps.append(m)
    return in_maps


def _run(x, w, b, trace=False):
    from concourse.bass_utils import run_bass_kernel_spmd

    nc = _get_program()
    in_maps = _prep_inputs(x, w, b)
    res = run_bass_kernel_spmd(nc, in_maps, core_ids=list(range(NCORES)), trace=trace)
    outs = [np.asarray(r["out"]) for r in res.results]  # each [ORDER, ROWS, N] f16
    full = np.concatenate(outs, axis=1)  # [ORDER, B, N]
    return (
        tuple(np.ascontiguousarray(full[i]).astype(np.float32) for i in range(ORDER)),
        res,
    )


def kernel(x, w, b):
    outs, _ = _run(x, w, b, trace=False)
    return outs
